# revision 30
# baseline (speedup 1.0000x reference)
"""Trainium2 Bass kernel for nn_DS_Block (topk_masking).

Split of work (see analysis in test.py):
- The argsort/top-k selection order on w1 is numerically chaotic: reference w1
  has 78 exact fp32 ties and hundreds of gaps < 1e-7, so even a perfect-fp64
  device recompute of w1 flips ~100 argsort positions vs the fp32 reference,
  which scrambles the gathered outputs (y_ds etc.) with O(1) errors. The only
  way to reproduce the reference selection exactly is to run the identical
  jnp ops on CPU (bit-exact, verified). So the selection chain + gathers run
  on host; they are cheap (~0.4 GFLOP of the ~9.2 GFLOP total).
- The FLOP-heavy part (per-sample 128x128 ResNet block over the 4096 selected
  points, instance norms, point weighting, and the 9x9 weighted Gram matrix
  = ~95% of FLOPs) runs on the 8 NeuronCores, data-parallel over the batch
  (4 batches per core).
- The 32 tiny 9x9 eigensolves run on host LAPACK via the same jnp.linalg.eigh
  the reference uses (keeps the eigenvector sign convention).
"""

import numpy as np

B = 32
N = 8192
ND = 4096          # n_ds = N * 0.5
NCORES = 8
BPC = B // NCORES  # batches per core
NCH = ND // 512    # 512-wide matmul chunks
NJ = ND // 128     # 128-wide chunks (points-on-partitions layout)
EPS = 1e-5

_STATE = {}

PSP_BUFS = 4
PSS_BUFS = 2
BIG_BUFS = 2
SM_BUFS = 2


def _build_nc():
    import concourse.bacc as bacc
    import concourse.tile as tile
    import concourse.mybir as mybir

    f32 = mybir.dt.float32
    f32r = mybir.dt.float32r
    i32 = mybir.dt.int32
    AF = mybir.ActivationFunctionType
    OP = mybir.AluOpType

    nc = bacc.Bacc("TRN2", target_bir_lowering=False, debug=False, num_devices=NCORES)

    xdsT = nc.dram_tensor("xdsT", [BPC, 4, ND], f32r, kind="ExternalInput")
    xdsP = nc.dram_tensor("xdsP", [BPC, 128, NJ, 4], f32, kind="ExternalInput")
    a0t = nc.dram_tensor("a0t", [4, 128], f32r, kind="ExternalInput")
    cvec = nc.dram_tensor("cvec", [128, 7], f32, kind="ExternalInput")
    l1wt = nc.dram_tensor("l1wt", [128, 128], f32r, kind="ExternalInput")
    l2wt = nc.dram_tensor("l2wt", [128, 128], f32r, kind="ExternalInput")
    lin2wt = nc.dram_tensor("lin2wt", [128, 2], f32, kind="ExternalInput")
    w2r0 = nc.dram_tensor("w2r0", [BPC, 128, NJ], f32, kind="ExternalOutput")
    wout = nc.dram_tensor("wout", [BPC, 128, NJ], f32, kind="ExternalOutput")
    xwx = nc.dram_tensor("xwx", [BPC, 9, 9], f32, kind="ExternalOutput")

    with tile.TileContext(nc) as tc:
        with (
            tc.tile_pool(name="wp", bufs=1) as wp,
            tc.tile_pool(name="big", bufs=BIG_BUFS) as bigp,
            tc.tile_pool(name="sm", bufs=SM_BUFS) as smp,
            tc.tile_pool(name="ps", bufs=PSP_BUFS, space="PSUM") as psp,
            tc.tile_pool(name="pss", bufs=PSS_BUFS, space="PSUM") as pss,
        ):
            a0t_sb = wp.tile([4, 128], f32r)
            nc.sync.dma_start(a0t_sb[:], a0t[:, :])
            cvec_sb = wp.tile([128, 7], f32)
            nc.sync.dma_start(cvec_sb[:], cvec[:, :])
            l1wt_sb = wp.tile([128, 128], f32r)
            nc.sync.dma_start(l1wt_sb[:], l1wt[:, :])
            l2wt_sb = wp.tile([128, 128], f32r)
            nc.sync.dma_start(l2wt_sb[:], l2wt[:, :])
            lin2wt_sb = wp.tile([128, 2], f32)
            nc.sync.dma_start(lin2wt_sb[:], lin2wt[:, :])


            def instnorm_coeffs(mv, gcol, bcol, tag):
                # scale = g * rsqrt(var+eps), bias = b - mean*scale.
                # rsqrt is done entirely on DVE (bit-trick seed + 3 Newton
                # steps) so the ACT engine never needs the Ln/Sqrt table sets:
                # the whole kernel then uses only exp_and_others (Relu, Copy,
                # Exp), i.e. a single ACT table load instead of 4 per batch.
                s = smp.tile([128, 1], f32, tag=f"s{tag}")
                bb = smp.tile([128, 1], f32, tag=f"b{tag}")
                tmp = smp.tile([128, 1], f32, tag=f"tmp{tag}")
                v = smp.tile([128, 1], f32, tag=f"v{tag}")
                vh = smp.tile([128, 1], f32, tag=f"vh{tag}")
                nc.vector.tensor_scalar_add(v[:], mv[:, 1:2], EPS)
                nc.vector.tensor_scalar_mul(vh[:], v[:], 0.5)
                # seed y = bitcast(0x5f3759df - (bitcast(v) >> 1))
                y = s  # build the rsqrt in-place in s
                nc.vector.tensor_scalar(
                    y[:].bitcast(i32),
                    v[:].bitcast(i32),
                    1,
                    scalar2=None,
                    op0=OP.logical_shift_right,
                )
                nc.vector.tensor_scalar(
                    y[:].bitcast(i32),
                    y[:].bitcast(i32),
                    -1,
                    scalar2=0x5F3759DF,
                    op0=OP.mult,
                    op1=OP.add,
                )
                for _ in range(3):  # y *= 1.5 - 0.5*v*y^2
                    nc.vector.tensor_mul(tmp[:], y[:], y[:])
                    nc.vector.tensor_mul(tmp[:], tmp[:], vh[:])
                    nc.vector.tensor_scalar(
                        tmp[:], tmp[:], -1.0, scalar2=1.5, op0=OP.mult, op1=OP.add
                    )
                    nc.vector.tensor_mul(y[:], y[:], tmp[:])
                nc.vector.tensor_mul(s[:], y[:], cvec_sb[:, gcol : gcol + 1])
                nc.vector.tensor_mul(tmp[:], mv[:, 0:1], s[:])
                nc.vector.tensor_sub(bb[:], cvec_sb[:, bcol : bcol + 1], tmp[:])
                return s, bb

            def emit_batch(b):
                xT = bigp.tile([4, ND], f32r, tag="xT")
                nc.sync.dma_start(xT[:], xdsT[b, :, :])
                xP = smp.tile([128, NJ, 4], f32, tag="xP")
                nc.sync.dma_start(xP[:], xdsP[b, :, :, :])

                # conv(4->128) + bn0 + relu, recomputed on the gathered points
                feat = bigp.tile([128, ND], f32r, tag="feat")
                for j in range(NCH):
                    sl = slice(j * 512, (j + 1) * 512)
                    ps = psp.tile([128, 512], f32, tag="mm")
                    nc.tensor.matmul(
                        ps[:], a0t_sb[:], xT[:, sl],
                        start=True, stop=True,
                    )
                    nc.scalar.activation(
                        feat[:, sl], ps[:], AF.Relu, bias=cvec_sb[:, 0:1], scale=1.0
                    )

                yield "conv"
                # l1 + instnorm stats (l1_b is cancelled by the instnorm mean)
                h1 = bigp.tile([128, ND], f32, tag="h")
                st1 = smp.tile([128, NCH, 6], f32, tag="st")
                for j in range(NCH):
                    sl = slice(j * 512, (j + 1) * 512)
                    ps = psp.tile([128, 512], f32, tag="mm")
                    nc.tensor.matmul(
                        ps[:], l1wt_sb[:], feat[:, sl],
                        start=True, stop=True,
                    )
                    nc.vector.bn_stats(st1[:, j, :], ps[:])
                    if j % 2 == 0:
                        nc.scalar.copy(h1[:, sl], ps[:])
                    else:
                        nc.vector.tensor_copy(h1[:, sl], ps[:])
                yield "l1"
                mv1 = smp.tile([128, 2], f32, tag="mv")
                nc.vector.bn_aggr(mv1[:], st1[:])
                s1, b1 = instnorm_coeffs(mv1, 1, 2, "1")
                r1 = bigp.tile([128, ND], f32r, tag="rt")
                for c in range(4):  # chunked so l2 matmuls can start early
                    cl = slice(c * 1024, (c + 1) * 1024)
                    nc.scalar.activation(r1[:, cl], h1[:, cl], AF.Relu, bias=b1[:], scale=s1[:])

                yield "relu1"
                # l2 + instnorm stats
                h2 = bigp.tile([128, ND], f32, tag="h")
                st2 = smp.tile([128, NCH, 6], f32, tag="st")
                for j in range(NCH):
                    sl = slice(j * 512, (j + 1) * 512)
                    ps = psp.tile([128, 512], f32, tag="mm")
                    nc.tensor.matmul(
                        ps[:], l2wt_sb[:], r1[:, sl],
                        start=True, stop=True,
                    )
                    nc.vector.bn_stats(st2[:, j, :], ps[:])
                    if j % 2 == 0:
                        nc.scalar.copy(h2[:, sl], ps[:])
                    else:
                        nc.vector.tensor_copy(h2[:, sl], ps[:])
                yield "l2"
                mv2 = smp.tile([128, 2], f32, tag="mv")
                nc.vector.bn_aggr(mv2[:], st2[:])
                s2, b2 = instnorm_coeffs(mv2, 3, 4, "2")

                # feat2 = relu(h2*s2 + b2 + feat)
                t = bigp.tile([128, ND], f32, tag="rt")
                for c in range(4):  # chunked pipeline: STT -> relu -> lin2
                    cl = slice(c * 1024, (c + 1) * 1024)
                    nc.vector.scalar_tensor_tensor(
                        t[:, cl], h2[:, cl], s2[:], feat[:, cl].bitcast(f32), op0=OP.mult, op1=OP.add
                    )
                    nc.scalar.activation(t[:, cl], t[:, cl], AF.Relu, bias=b2[:], scale=1.0)

                yield "resid"
                # lin2 in points-on-partitions layout: out[n,2] per 128-chunk
                psw = pss.tile([128, NJ, 2], f32, tag="psw")
                for j in range(NJ):
                    nc.tensor.matmul(
                        psw[:, j, :],
                        t[:, j * 128 : (j + 1) * 128],
                        lin2wt_sb[:],
                        start=(j == 0),
                        stop=(j == NJ - 1),
                        skip_group_check=True,
                    )
                w2t = smp.tile([128, NJ, 2], f32, tag="w2t")
                nc.vector.tensor_copy(w2t[:], psw[:])
                nc.sync.dma_start(w2r0[b, :, :], w2t[:, :, 0])

                yield "lin2"
                # w = exp(w2_1+c1) * sigmoid(w2_0+c0) = exp(w2_1+c1) / (1 + exp(-w2_0-c0))
                # cvec col 5 = -lin2_b[0], col 6 = +lin2_b[1]
                e0 = smp.tile([128, NJ], f32, tag="e0")
                nc.scalar.activation(
                    e0[:], w2t[:, :, 0], AF.Exp, bias=cvec_sb[:, 5:6], scale=-1.0
                )
                e1 = smp.tile([128, NJ], f32, tag="e1")
                nc.scalar.activation(
                    e1[:], w2t[:, :, 1], AF.Exp, bias=cvec_sb[:, 6:7], scale=1.0
                )
                nc.vector.tensor_scalar_add(e0[:], e0[:], 1.0)
                nc.vector.reciprocal(e0[:], e0[:])  # e0 = sigmoid(w2_0+c0)
                wv = smp.tile([128, NJ], f32, tag="wv")
                nc.vector.tensor_mul(wv[:], e1[:], e0[:])
                nc.sync.dma_start(wout[b, :, :], wv[:])

                yield "wchain"
                # X = [a2a0, a2a1, a2, a3a0, a3a1, a3, a0, a1, 1]  [128, NJ, 9]
                X = smp.tile([128, NJ, 9], f32, tag="X")
                a0v, a1v, a2v, a3v = (xP[:, :, c] for c in range(4))
                nc.gpsimd.tensor_mul(X[:, :, 0], a2v, a0v)
                nc.gpsimd.tensor_mul(X[:, :, 1], a2v, a1v)
                nc.gpsimd.tensor_copy(X[:, :, 2], a2v)
                nc.gpsimd.tensor_mul(X[:, :, 3], a3v, a0v)
                nc.gpsimd.tensor_mul(X[:, :, 4], a3v, a1v)
                nc.gpsimd.tensor_copy(X[:, :, 5], a3v)
                nc.gpsimd.tensor_copy(X[:, :, 6], a0v)
                nc.gpsimd.tensor_copy(X[:, :, 7], a1v)
                nc.gpsimd.memset(X[:, :, 8], 1.0)
                wX = smp.tile([128, NJ, 9], f32, tag="wX")
                for i in range(9):
                    nc.gpsimd.tensor_mul(wX[:, :, i], X[:, :, i], wv[:])

                yield "X"
                # XwX[9,9] = sum_n w_n X_n X_n^T, accumulated over NJ chunks
                ps9 = pss.tile([9, 9], f32, tag="ps9")
                for j in range(NJ):
                    nc.tensor.matmul(
                        ps9[:],
                        wX[:, j, :],
                        X[:, j, :],
                        start=(j == 0),
                        stop=(j == NJ - 1),
                    )
                xw_sb = smp.tile([9, 9], f32, tag="xw")
                nc.vector.tensor_copy(xw_sb[:], ps9[:])
                nc.sync.dma_start(xwx[b, :, :], xw_sb[:])
                yield "done"

            # Sequential emission (stage interleaving measured slower on the
            # timeline model: PSUM contention between paired batches).
            for bb in range(BPC):
                for _ in emit_batch(bb):
                    pass

    nc.finalize()
    return nc


def _get_nc():
    if "nc" not in _STATE:
        _STATE["nc"] = _build_nc()
    return _STATE["nc"]


def kernel(**inputs):
    import os

    import jax
    import jax.numpy as jnp
    from concourse.bass_utils import run_bass_kernel_spmd

    # The axon client in this container has no NTFF profiling hook; a stray
    # BASS_TRACE=1 in the environment would crash run_bass_kernel_spmd on an
    # import of antenv.axon_hooks. Force tracing off.
    os.environ["BASS_NEVER_TRACE"] = "1"

    x = np.asarray(inputs["x"])
    y = np.asarray(inputs["y"])
    conv_w = np.asarray(inputs["conv_w"])
    conv_b = np.asarray(inputs["conv_b"])
    bn0_g = np.asarray(inputs["bn0_g"])
    bn0_b = np.asarray(inputs["bn0_b"])
    lin0_w = np.asarray(inputs["lin0_w"])
    lin0_b = np.asarray(inputs["lin0_b"])
    lin1_w = np.asarray(inputs["lin1_w"])
    lin1_b = np.asarray(inputs["lin1_b"])
    l1_w = np.asarray(inputs["l1_w"])
    bn1_g = np.asarray(inputs["bn1_g"])
    bn1_b = np.asarray(inputs["bn1_b"])
    l2_w = np.asarray(inputs["l2_w"])
    bn2_g = np.asarray(inputs["bn2_g"])
    bn2_b = np.asarray(inputs["bn2_b"])
    lin2_w = np.asarray(inputs["lin2_w"])
    lin2_b = np.asarray(inputs["lin2_b"])

    cpu = jax.devices("cpu")[0]
    # Host selection chain: identical jnp ops to the reference, on CPU, so the
    # argsort keys (and hence the selection order, including exact fp32 ties)
    # match the reference bit-for-bit.
    with jax.default_device(cpu):
        x0 = jnp.asarray(x)[:, 0]
        feat = jnp.einsum("oc,bnc->bon", jnp.asarray(conv_w), x0) + conv_b[None, :, None]
        feat = jax.nn.relu(feat * bn0_g[None, :, None] + bn0_b[None, :, None])
        w0 = jnp.einsum("oc,bcn->bon", jnp.asarray(lin0_w), feat)[:, 0] + lin0_b[0]
        w1 = jnp.einsum("oc,bcn->bon", jnp.asarray(lin1_w), feat)[:, 0] + lin1_b[0]
        order = jnp.argsort(-w1, axis=-1)
        w1_sorted = jnp.take_along_axis(w1, order, axis=-1)
        idx = order[:, :ND]
        w1_ds = w1_sorted[:, :ND]
        y_ds = jnp.take_along_axis(jnp.asarray(y), idx, axis=-1)
        w0_ds = jnp.take_along_axis(w0, idx, axis=-1)
        x_ds = jnp.take_along_axis(x0, idx[:, :, None], axis=1)

    w0 = np.asarray(w0)
    w1 = np.asarray(w1)
    w0_ds = np.asarray(w0_ds)
    w1_ds = np.asarray(w1_ds)
    y_ds = np.asarray(y_ds)
    x_ds = np.asarray(x_ds)  # [B, ND, 4]

    # Device inputs
    xdsT = np.ascontiguousarray(x_ds.transpose(0, 2, 1))  # [B, 4, ND]
    xdsP = np.ascontiguousarray(
        x_ds.reshape(B, NJ, 128, 4).transpose(0, 2, 1, 3)
    )  # [B, 128, NJ, 4]
    a0 = bn0_g[:, None] * conv_w  # [128, 4]
    a0t = np.ascontiguousarray(a0.T)  # [4, 128]
    b0 = bn0_g * conv_b + bn0_b
    ones = np.ones(128, dtype=np.float32)
    cvec = np.ascontiguousarray(
        np.stack(
            [b0, bn1_g, bn1_b, bn2_g, bn2_b, -lin2_b[0] * ones, lin2_b[1] * ones],
            axis=1,
        )
    )  # [128, 7]
    l1wt = np.ascontiguousarray(l1_w.T)
    l2wt = np.ascontiguousarray(l2_w.T)
    lin2wt = np.ascontiguousarray(lin2_w.T)  # [128, 2]

    nc = _get_nc()
    in_maps = []
    for c in range(NCORES):
        bs = slice(c * BPC, (c + 1) * BPC)
        in_maps.append(
            {
                "xdsT": np.ascontiguousarray(xdsT[bs]),
                "xdsP": np.ascontiguousarray(xdsP[bs]),
                "a0t": a0t,
                "cvec": cvec,
                "l1wt": l1wt,
                "l2wt": l2wt,
                "lin2wt": lin2wt,
            }
        )
    res = run_bass_kernel_spmd(nc, in_maps, core_ids=list(range(NCORES)))
    _STATE["last_result"] = res

    w2r0_dev = np.concatenate([r["w2r0"] for r in res.results], axis=0)  # [B,128,NJ]
    w_dev = np.concatenate([r["wout"] for r in res.results], axis=0)  # [B,128,NJ]
    xwx_dev = np.concatenate([r["xwx"] for r in res.results], axis=0)  # [B,9,9]

    # n = j*128 + p  ->  [B, NJ, 128] -> [B, ND]
    w2_0 = w2r0_dev.transpose(0, 2, 1).reshape(B, ND) + lin2_b[0]
    w_full = w_dev.transpose(0, 2, 1).reshape(B, ND)
    wsum = w_full.sum(axis=-1)
    # jnp.linalg.eigh symmetrizes its input, matching what the reference's
    # eigh call does to its (slightly asymmetric) fp32 einsum result.
    xwx_n = xwx_dev / (wsum + 1e-5)[:, None, None]

    with jax.default_device(cpu):
        _, v = jnp.linalg.eigh(jnp.asarray(xwx_n))
        e_hat = v[:, :, 0]
        e_hat = e_hat / jnp.linalg.norm(e_hat, axis=1, keepdims=True)
    e_hat = np.asarray(e_hat)

    x_ds_out = x_ds[:, None, :, :]  # [B, 1, ND, 4]
    return (x_ds_out, y_ds, w0, w1, w2_0, w0_ds, w1_ds, e_hat)


# revision 44
# speedup vs baseline: 1.3116x; 1.3116x over previous
"""Trainium2 Bass kernel for nn_DS_Block (topk_masking).

Split of work (see analysis in test.py):
- The argsort/top-k selection order on w1 is numerically chaotic: reference w1
  has 78 exact fp32 ties and hundreds of gaps < 1e-7, so even a perfect-fp64
  device recompute of w1 flips ~100 argsort positions vs the fp32 reference,
  which scrambles the gathered outputs (y_ds etc.) with O(1) errors. The only
  way to reproduce the reference selection exactly is to run the identical
  jnp ops on CPU (bit-exact, verified). So the selection chain + gathers run
  on host; they are cheap (~0.4 GFLOP of the ~9.2 GFLOP total).
- The FLOP-heavy part (per-sample 128x128 ResNet block over the 4096 selected
  points, instance norms, point weighting, and the 9x9 weighted Gram matrix
  = ~95% of FLOPs) runs on the 8 NeuronCores, data-parallel over the batch
  (4 batches per core).
- The 32 tiny 9x9 eigensolves run on host LAPACK via the same jnp.linalg.eigh
  the reference uses (keeps the eigenvector sign convention).
"""

import numpy as np

B = 32
N = 8192
ND = 4096          # n_ds = N * 0.5
NCORES = 8
BPC = B // NCORES  # batches per core
NCH = ND // 512    # 512-wide matmul chunks
NJ = ND // 128     # 128-wide chunks (points-on-partitions layout)
EPS = 1e-5

_STATE = {}

PSP_BUFS = 4
PSS_BUFS = 2
BIG_BUFS = 2
SM_BUFS = 2


def _build_nc():
    import concourse.bacc as bacc
    import concourse.tile as tile
    import concourse.mybir as mybir

    f32 = mybir.dt.float32
    f32r = mybir.dt.float32r
    i32 = mybir.dt.int32
    AF = mybir.ActivationFunctionType
    OP = mybir.AluOpType

    nc = bacc.Bacc("TRN2", target_bir_lowering=False, debug=False, num_devices=NCORES)

    xdsT = nc.dram_tensor("xdsT", [BPC, 4, ND], f32r, kind="ExternalInput")
    xdsP = nc.dram_tensor("xdsP", [BPC, 128, NJ, 4], f32, kind="ExternalInput")
    a0t = nc.dram_tensor("a0t", [4, 128], f32r, kind="ExternalInput")
    cvec = nc.dram_tensor("cvec", [128, 7], f32, kind="ExternalInput")
    l1wt = nc.dram_tensor("l1wt", [128, 128], f32r, kind="ExternalInput")
    l2wt = nc.dram_tensor("l2wt", [128, 128], f32r, kind="ExternalInput")
    lin2wt = nc.dram_tensor("lin2wt", [128, 2], f32, kind="ExternalInput")
    w2r0 = nc.dram_tensor("w2r0", [BPC, 128, NJ], f32, kind="ExternalOutput")
    wout = nc.dram_tensor("wout", [BPC, 128, NJ], f32, kind="ExternalOutput")
    xwx = nc.dram_tensor("xwx", [BPC, 9, 9], f32, kind="ExternalOutput")

    with tile.TileContext(nc) as tc:
        with (
            tc.tile_pool(name="wp", bufs=1) as wp,
            tc.tile_pool(name="big", bufs=BIG_BUFS) as bigp,
            tc.tile_pool(name="sm", bufs=SM_BUFS) as smp,
            tc.tile_pool(name="ps", bufs=PSP_BUFS, space="PSUM") as psp,
            tc.tile_pool(name="pss", bufs=PSS_BUFS, space="PSUM") as pss,
        ):
            a0t_sb = wp.tile([4, 128], f32r)
            nc.sync.dma_start(a0t_sb[:], a0t[:, :])
            cvec_sb = wp.tile([128, 7], f32)
            nc.sync.dma_start(cvec_sb[:], cvec[:, :])
            l1wt_sb = wp.tile([128, 128], f32r)
            nc.sync.dma_start(l1wt_sb[:], l1wt[:, :])
            l2wt_sb = wp.tile([128, 128], f32r)
            nc.sync.dma_start(l2wt_sb[:], l2wt[:, :])
            lin2wt_sb = wp.tile([128, 2], f32)
            nc.sync.dma_start(lin2wt_sb[:], lin2wt[:, :])

            def instnorm_coeffs(mv, gcol, bcol, tag):
                # scale = g * rsqrt(var+eps), bias = b - mean*scale.
                # rsqrt is done entirely on DVE (bit-trick seed + 3 Newton
                # steps) so the ACT engine never needs the Ln/Sqrt table sets:
                # the whole kernel then uses only exp_and_others (Relu, Copy,
                # Exp), i.e. a single ACT table load instead of 4 per batch.
                s = smp.tile([128, 1], f32, tag=f"s{tag}")
                bb = smp.tile([128, 1], f32, tag=f"b{tag}")
                tmp = smp.tile([128, 1], f32, tag=f"tmp{tag}")
                v = smp.tile([128, 1], f32, tag=f"v{tag}")
                vh = smp.tile([128, 1], f32, tag=f"vh{tag}")
                nc.vector.tensor_scalar_add(v[:], mv[:, 1:2], EPS)
                nc.vector.tensor_scalar_mul(vh[:], v[:], -0.5)  # -v/2
                # seed y = bitcast(0x5f3759df - (bitcast(v) >> 1))
                y = s  # build the rsqrt in-place in s
                nc.vector.tensor_scalar(
                    y[:].bitcast(i32),
                    v[:].bitcast(i32),
                    1,
                    scalar2=None,
                    op0=OP.logical_shift_right,
                )
                nc.vector.tensor_scalar(
                    y[:].bitcast(i32),
                    y[:].bitcast(i32),
                    -1,
                    scalar2=0x5F3759DF,
                    op0=OP.mult,
                    op1=OP.add,
                )
                for _ in range(3):  # y *= 1.5 + (-v/2)*y^2, 3 ops per step
                    nc.vector.tensor_mul(tmp[:], y[:], y[:])
                    nc.vector.tensor_scalar(
                        tmp[:], tmp[:], vh[:], scalar2=1.5, op0=OP.mult, op1=OP.add
                    )
                    nc.vector.tensor_mul(y[:], y[:], tmp[:])
                nc.vector.tensor_mul(s[:], y[:], cvec_sb[:, gcol : gcol + 1])
                nc.vector.tensor_mul(tmp[:], mv[:, 0:1], s[:])
                nc.vector.tensor_sub(bb[:], cvec_sb[:, bcol : bcol + 1], tmp[:])
                return s, bb

            def emit_batch(b):
                xT = bigp.tile([4, ND], f32r, tag="xT")
                nc.sync.dma_start(xT[:], xdsT[b, :, :])
                xP = smp.tile([128, NJ, 4], f32, tag="xP")
                nc.sync.dma_start(xP[:], xdsP[b, :, :, :])

                # conv(4->128) + bn0 + relu, recomputed on the gathered points
                feat = bigp.tile([128, ND], f32r, tag="feat")
                for j in range(NCH):
                    sl = slice(j * 512, (j + 1) * 512)
                    ps = psp.tile([128, 512], f32, tag="mm")
                    nc.tensor.matmul(
                        ps[:], a0t_sb[:], xT[:, sl],
                        start=True, stop=True,
                    )
                    if j < N_CONV_DVE:
                        nc.vector.tensor_scalar(
                            feat[:, sl], ps[:], cvec_sb[:, 0:1], scalar2=0.0,
                            op0=OP.add, op1=OP.max,
                        )
                    else:
                        nc.scalar.activation(
                            feat[:, sl], ps[:], AF.Relu, bias=cvec_sb[:, 0:1], scale=1.0
                        )

                yield "conv"
                # l1 + instnorm stats (l1_b is cancelled by the instnorm mean)
                h1 = bigp.tile([128, ND], f32, tag="h")
                st1 = smp.tile([128, NCH, 6], f32, tag="st")
                for j in range(NCH):
                    sl = slice(j * 512, (j + 1) * 512)
                    ps = psp.tile([128, 512], f32, tag="mm")
                    nc.tensor.matmul(
                        ps[:], l1wt_sb[:], feat[:, sl],
                        start=True, stop=True,
                    )
                    if j % 2 == 0:
                        nc.scalar.copy(h1[:, sl], ps[:])
                    else:
                        nc.vector.tensor_copy(h1[:, sl], ps[:])
                    # stats read the SBUF copy: DVE SBUF access is 58 cycles
                    # vs 120 for PSUM
                    nc.vector.bn_stats(st1[:, j, :], h1[:, sl])
                yield "l1"
                mv1 = smp.tile([128, 2], f32, tag="mv")
                nc.vector.bn_aggr(mv1[:], st1[:])
                s1, b1 = instnorm_coeffs(mv1, 1, 2, "1")
                r1 = bigp.tile([128, ND], f32r, tag="rt")
                for c in range(8):  # chunked so l2 matmuls can start early
                    cl = slice(c * 512, (c + 1) * 512)
                    if c < N_RELU1_DVE:
                        # 2-op DVE equivalent fills DVE's stall during this leg
                        nc.vector.tensor_scalar(
                            r1[:, cl], h1[:, cl], s1[:], scalar2=b1[:],
                            op0=OP.mult, op1=OP.add,
                        )
                        nc.vector.tensor_scalar_max(r1[:, cl], r1[:, cl], 0.0)
                    else:
                        nc.scalar.activation(r1[:, cl], h1[:, cl], AF.Relu, bias=b1[:], scale=s1[:])

                yield "relu1"
                # l2 + instnorm stats
                h2 = bigp.tile([128, ND], f32, tag="h")
                st2 = smp.tile([128, NCH, 6], f32, tag="st")
                for j in range(NCH):
                    sl = slice(j * 512, (j + 1) * 512)
                    ps = psp.tile([128, 512], f32, tag="mm")
                    nc.tensor.matmul(
                        ps[:], l2wt_sb[:], r1[:, sl],
                        start=True, stop=True,
                    )
                    if j % 2 == 0:
                        nc.scalar.copy(h2[:, sl], ps[:])
                    else:
                        nc.vector.tensor_copy(h2[:, sl], ps[:])
                    nc.vector.bn_stats(st2[:, j, :], h2[:, sl])
                yield "l2"
                mv2 = smp.tile([128, 2], f32, tag="mv")
                nc.vector.bn_aggr(mv2[:], st2[:])
                s2, b2 = instnorm_coeffs(mv2, 3, 4, "2")

                # feat2 = relu(h2*s2 + b2 + feat)
                t = bigp.tile([128, ND], f32, tag="rt")
                for c in range(8):  # chunked pipeline: STT -> relu -> lin2
                    cl = slice(c * 512, (c + 1) * 512)
                    nc.vector.scalar_tensor_tensor(
                        t[:, cl], h2[:, cl], s2[:], feat[:, cl].bitcast(f32), op0=OP.mult, op1=OP.add
                    )
                    if c < N_RELU2_DVE:
                        nc.vector.tensor_scalar(
                            t[:, cl], t[:, cl], b2[:], scalar2=0.0, op0=OP.add, op1=OP.max
                        )
                    else:
                        nc.scalar.activation(t[:, cl], t[:, cl], AF.Relu, bias=b2[:], scale=1.0)

                yield "resid"
                # lin2 in points-on-partitions layout: out[n,2] per 128-chunk
                psw = pss.tile([128, NJ, 2], f32, tag="pshared")
                for j in range(NJ):
                    nc.tensor.matmul(
                        psw[:, j, :],
                        t[:, j * 128 : (j + 1) * 128],
                        lin2wt_sb[:],
                        start=(j == 0),
                        stop=(j == NJ - 1),
                        skip_group_check=True,
                    )
                w2t = smp.tile([128, NJ, 2], f32, tag="w2t")
                nc.scalar.copy(w2t[:], psw[:])
                nc.sync.dma_start(w2r0[b, :, :], w2t[:, :, 0])

                yield "lin2"
                # w = exp(w2_1+c1) * sigmoid(w2_0+c0) = exp(w2_1+c1) / (1 + exp(-w2_0-c0))
                # cvec col 5 = -lin2_b[0], col 6 = +lin2_b[1]
                e0 = smp.tile([128, NJ], f32, tag="e0")
                nc.scalar.activation(
                    e0[:], w2t[:, :, 0], AF.Exp, bias=cvec_sb[:, 5:6], scale=-1.0
                )
                e1 = smp.tile([128, NJ], f32, tag="e1")
                nc.scalar.activation(
                    e1[:], w2t[:, :, 1], AF.Exp, bias=cvec_sb[:, 6:7], scale=1.0
                )
                nc.vector.tensor_scalar_add(e0[:], e0[:], 1.0)
                nc.vector.reciprocal(e0[:], e0[:])  # e0 = sigmoid(w2_0+c0)
                wv = smp.tile([128, NJ], f32, tag="wv")
                nc.vector.tensor_mul(wv[:], e1[:], e0[:])
                nc.sync.dma_start(wout[b, :, :], wv[:])

                yield "wchain"
                # X = [a2a0, a2a1, a2, a3a0, a3a1, a3, a0, a1, 1]  [128, NJ, 9]
                X = smp.tile([128, NJ, 9], f32, tag="X")
                a0v, a1v, a2v, a3v = (xP[:, :, c] for c in range(4))
                nc.gpsimd.tensor_mul(X[:, :, 0], a2v, a0v)
                nc.gpsimd.tensor_mul(X[:, :, 1], a2v, a1v)
                nc.gpsimd.tensor_copy(X[:, :, 2], a2v)
                nc.gpsimd.tensor_mul(X[:, :, 3], a3v, a0v)
                nc.gpsimd.tensor_mul(X[:, :, 4], a3v, a1v)
                nc.gpsimd.tensor_copy(X[:, :, 5], a3v)
                nc.gpsimd.tensor_copy(X[:, :, 6], a0v)
                nc.gpsimd.tensor_copy(X[:, :, 7], a1v)
                nc.gpsimd.memset(X[:, :, 8], 1.0)
                wX = smp.tile([128, NJ, 9], f32, tag="wX")
                for i in range(9):
                    nc.gpsimd.tensor_mul(wX[:, :, i], X[:, :, i], wv[:])

                yield "X"
                # XwX[9,9] = sum_n w_n X_n X_n^T, accumulated over NJ chunks
                ps9 = pss.tile([9, 9], f32, tag="pshared")
                for j in range(NJ):
                    nc.tensor.matmul(
                        ps9[:],
                        wX[:, j, :],
                        X[:, j, :],
                        start=(j == 0),
                        stop=(j == NJ - 1),
                    )
                xw_sb = smp.tile([9, 9], f32, tag="xw")
                nc.scalar.copy(xw_sb[:], ps9[:])
                nc.sync.dma_start(xwx[b, :, :], xw_sb[:])
                yield "done"

            # Staggered emission: batch b's X/wX/XwX tail is emitted after
            # batch b+1's main body. Per-engine instruction streams are static
            # in program order, so emitting xwx(b) (which waits on the whole
            # w-chain) before conv(b+1) head-of-line-blocks the PE stream and
            # stalls every engine ~4us at each batch boundary (seen in the
            # timeline trace). Deferring the tail removes that stall.
            prev_tail = None
            for bb in range(BPC):
                g = emit_batch(bb)
                for stage in g:
                    if stage == "wchain":
                        break
                if prev_tail is not None:
                    for _ in prev_tail:
                        pass
                prev_tail = g
            if prev_tail is not None:
                for _ in prev_tail:
                    pass

    nc.finalize()
    return nc


def _get_nc():
    if "nc" not in _STATE:
        _STATE["nc"] = _build_nc()
    return _STATE["nc"]


def kernel(**inputs):
    import os

    import jax
    import jax.numpy as jnp
    from concourse.bass_utils import run_bass_kernel_spmd

    # The axon client in this container has no NTFF profiling hook; a stray
    # BASS_TRACE=1 in the environment would crash run_bass_kernel_spmd on an
    # import of antenv.axon_hooks. Force tracing off.
    os.environ["BASS_NEVER_TRACE"] = "1"

    x = np.asarray(inputs["x"])
    y = np.asarray(inputs["y"])
    conv_w = np.asarray(inputs["conv_w"])
    conv_b = np.asarray(inputs["conv_b"])
    bn0_g = np.asarray(inputs["bn0_g"])
    bn0_b = np.asarray(inputs["bn0_b"])
    lin0_w = np.asarray(inputs["lin0_w"])
    lin0_b = np.asarray(inputs["lin0_b"])
    lin1_w = np.asarray(inputs["lin1_w"])
    lin1_b = np.asarray(inputs["lin1_b"])
    l1_w = np.asarray(inputs["l1_w"])
    bn1_g = np.asarray(inputs["bn1_g"])
    bn1_b = np.asarray(inputs["bn1_b"])
    l2_w = np.asarray(inputs["l2_w"])
    bn2_g = np.asarray(inputs["bn2_g"])
    bn2_b = np.asarray(inputs["bn2_b"])
    lin2_w = np.asarray(inputs["lin2_w"])
    lin2_b = np.asarray(inputs["lin2_b"])

    cpu = jax.devices("cpu")[0]
    # Host selection chain: identical jnp ops to the reference, on CPU, so the
    # argsort keys (and hence the selection order, including exact fp32 ties)
    # match the reference bit-for-bit.
    with jax.default_device(cpu):
        x0 = jnp.asarray(x)[:, 0]
        feat = jnp.einsum("oc,bnc->bon", jnp.asarray(conv_w), x0) + conv_b[None, :, None]
        feat = jax.nn.relu(feat * bn0_g[None, :, None] + bn0_b[None, :, None])
        w0 = jnp.einsum("oc,bcn->bon", jnp.asarray(lin0_w), feat)[:, 0] + lin0_b[0]
        w1 = jnp.einsum("oc,bcn->bon", jnp.asarray(lin1_w), feat)[:, 0] + lin1_b[0]
        order = jnp.argsort(-w1, axis=-1)
        w1_sorted = jnp.take_along_axis(w1, order, axis=-1)
        idx = order[:, :ND]
        w1_ds = w1_sorted[:, :ND]
        y_ds = jnp.take_along_axis(jnp.asarray(y), idx, axis=-1)
        w0_ds = jnp.take_along_axis(w0, idx, axis=-1)
        x_ds = jnp.take_along_axis(x0, idx[:, :, None], axis=1)

    w0 = np.asarray(w0)
    w1 = np.asarray(w1)
    w0_ds = np.asarray(w0_ds)
    w1_ds = np.asarray(w1_ds)
    y_ds = np.asarray(y_ds)
    x_ds = np.asarray(x_ds)  # [B, ND, 4]

    # Device inputs
    xdsT = np.ascontiguousarray(x_ds.transpose(0, 2, 1))  # [B, 4, ND]
    xdsP = np.ascontiguousarray(
        x_ds.reshape(B, NJ, 128, 4).transpose(0, 2, 1, 3)
    )  # [B, 128, NJ, 4]
    a0 = bn0_g[:, None] * conv_w  # [128, 4]
    a0t = np.ascontiguousarray(a0.T)  # [4, 128]
    b0 = bn0_g * conv_b + bn0_b
    ones = np.ones(128, dtype=np.float32)
    cvec = np.ascontiguousarray(
        np.stack(
            [b0, bn1_g, bn1_b, bn2_g, bn2_b, -lin2_b[0] * ones, lin2_b[1] * ones],
            axis=1,
        )
    )  # [128, 7]
    l1wt = np.ascontiguousarray(l1_w.T)
    l2wt = np.ascontiguousarray(l2_w.T)
    lin2wt = np.ascontiguousarray(lin2_w.T)  # [128, 2]

    nc = _get_nc()
    in_maps = []
    for c in range(NCORES):
        bs = slice(c * BPC, (c + 1) * BPC)
        in_maps.append(
            {
                "xdsT": np.ascontiguousarray(xdsT[bs]),
                "xdsP": np.ascontiguousarray(xdsP[bs]),
                "a0t": a0t,
                "cvec": cvec,
                "l1wt": l1wt,
                "l2wt": l2wt,
                "lin2wt": lin2wt,
            }
        )
    res = run_bass_kernel_spmd(nc, in_maps, core_ids=list(range(NCORES)))
    _STATE["last_result"] = res

    w2r0_dev = np.concatenate([r["w2r0"] for r in res.results], axis=0)  # [B,128,NJ]
    w_dev = np.concatenate([r["wout"] for r in res.results], axis=0)  # [B,128,NJ]
    xwx_dev = np.concatenate([r["xwx"] for r in res.results], axis=0)  # [B,9,9]

    # n = j*128 + p  ->  [B, NJ, 128] -> [B, ND]
    w2_0 = w2r0_dev.transpose(0, 2, 1).reshape(B, ND) + lin2_b[0]
    w_full = w_dev.transpose(0, 2, 1).reshape(B, ND)
    wsum = w_full.sum(axis=-1)
    # jnp.linalg.eigh symmetrizes its input, matching what the reference's
    # eigh call does to its (slightly asymmetric) fp32 einsum result.
    xwx_n = xwx_dev / (wsum + 1e-5)[:, None, None]

    with jax.default_device(cpu):
        _, v = jnp.linalg.eigh(jnp.asarray(xwx_n))
        e_hat = v[:, :, 0]
        e_hat = e_hat / jnp.linalg.norm(e_hat, axis=1, keepdims=True)
    e_hat = np.asarray(e_hat)

    x_ds_out = x_ds[:, None, :, :]  # [B, 1, ND, 4]
    return (x_ds_out, y_ds, w0, w1, w2_0, w0_ds, w1_ds, e_hat)

